# revision 12
# baseline (speedup 1.0000x reference)
"""3-layer GAT (heads=4, hid=128, concat=False) + mean-pool + MLP on 8 TRN2 cores.

Strategy: dst-shard nodes/edges across 8 cores. Per layer:
  - replicated matmul h = x @ Wcat on every core (x^T stationary, fp16)
  - h-table [NP, 576] fp16 rows [h0|1|h1|1|h2|1|h3|1|es(4xf32)|pad] written to HBM
  - ed-table [NP, 64] f32 rows [ed(4)|garbage] written to HBM
  - edges sorted by dst, tiled 128-dst/tile, B 128-edge blocks per tile (padded)
  - dma_gather h-rows by src (1152B elems) + ed rows by dst (256B elems)
  - w = exp(leaky_relu(es_src + ed_dst)); incidence matmul per head:
      lhsT = (dst_col == iota) * w_h  [128e x 128d],  rhs = g_h|1  [128e x 129]
      accumulated in PSUM over B blocks -> [128d, 129] = [agg | denom]
  - out = relu(sum_h agg_h/(4*denom_h) + b); AllGather relu_out for next layer
Pooling: per-graph segment matmul w/ host-built onehot, AllReduce, MLP replicated.
"""
import sys
import numpy as np

sys.path.insert(0, "/opt/trn_rl_repo")

N = 20000
E0 = 320000
G = 64
C = 8
SH = N // C            # 2500 real nodes per core
SHP = 2560             # padded shard (20 tiles x 128)
NP = C * SHP           # 20480 global padded rows
NT = NP // 128         # 160 node tiles
TL = SHP // 128        # 20 local dst tiles per core
HEADS = 4
HID = 128
ROW = 640              # h-table row fp16 elems: 4*129 + 8 + pad (1280B %256==0)
ED_ROW = 64            # ed-table row f32 elems (256B)
CH_BLK = 8             # gather chunk = 8 blocks = 1024 idxs
PAD_COL = 999.0


def _glob(n):
    return (n // SH) * SHP + (n % SH)


def _wrap_idx16(idx):
    """[E] -> [128, E/16] int16 gather-index layout (wrap 16, replicate x8)."""
    assert len(idx) % 16 == 0
    a = np.asarray(idx, np.int16).reshape(-1, 16).T  # [16, E/16]
    return np.tile(a, (8, 1)).copy()


def _host_prep(x, edge_index, batch, batch_size_tensor, Ws, MLP):
    (W1, as1, ad1, b1), (W2, as2, ad2, b2), (W3, as3, ad3, b3) = Ws
    M1, mb1, M2, mb2, M3, mb3 = MLP

    src = np.concatenate([np.asarray(edge_index[0], np.int64), np.arange(N)])
    dst = np.concatenate([np.asarray(edge_index[1], np.int64), np.arange(N)])
    order = np.argsort(dst, kind="stable")
    src, dst = src[order], dst[order]

    # per-core, per-tile edge lists
    core_of = dst // SH
    dloc = dst % SH
    tile_of = dloc // 128
    col_of = dloc % 128
    per = [[[] for _ in range(TL)] for _ in range(C)]
    for k in range(C):
        m = core_of == k
        s_k, t_k, c_k = src[m], tile_of[m], col_of[m]
        for e in range(len(s_k)):
            per[k][t_k[e]].append((s_k[e], c_k[e]))
    bmax = max(max((len(per[k][t]) for t in range(TL)), default=0) for k in range(C))
    B = -(-bmax // 128)
    B = -(-B // 4) * 4  # multiple of 4 so 20*B % 16 == 0
    EPAD = TL * B * 128
    NCH = TL * B // CH_BLK

    idx_main = np.zeros((C, 128, EPAD // 16), np.int16)
    idx_ed = np.zeros((C, 128, EPAD // 16), np.int16)
    dcol = np.zeros((C, 128, TL * B), np.float16)
    pool1h = np.zeros((C, 128, TL, G), np.float16)
    for k in range(C):
        sm = np.full(EPAD, NP - 1, np.int64)
        dm = np.full(EPAD, NP - 1, np.int64)
        cm = np.full(EPAD, PAD_COL, np.float64)
        for t in range(TL):
            lst = per[k][t]
            base = t * B * 128
            for i, (s, c) in enumerate(lst):
                sm[base + i] = _glob(s)
                dm[base + i] = SHP * k + t * 128 + c
                cm[base + i] = c
        idx_main[k] = _wrap_idx16(sm)
        idx_ed[k] = _wrap_idx16(dm)
        dcol[k] = cm.reshape(-1, 128).T.astype(np.float16)
        for t in range(TL):
            for p in range(128):
                mloc = t * 128 + p
                if mloc < SH:
                    pool1h[k, p, t, batch[k * SH + mloc]] = 1.0

    # x^T padded+fp16: [16, NP]
    xT = np.zeros((16, NP), np.float16)
    xg = np.zeros((NP, 12), np.float32)
    for k in range(C):
        xg[k * SHP : k * SHP + SH] = x[k * SH : (k + 1) * SH]
    xT[:12] = xg.T.astype(np.float16)

    def wcat(W, a_s, a_d, pad_k):
        din = W.shape[0]
        A_es = np.zeros((512, 4), np.float64)
        A_ed = np.zeros((512, 4), np.float64)
        for h in range(HEADS):
            A_es[h * 128 : (h + 1) * 128, h] = a_s[h]
            A_ed[h * 128 : (h + 1) * 128, h] = a_d[h]
        w = np.zeros((pad_k, 520), np.float32)
        w[:din, :512] = W
        w[:din, 512:516] = (W.astype(np.float64) @ A_es).astype(np.float32)
        w[:din, 516:520] = (W.astype(np.float64) @ A_ed).astype(np.float32)
        return w.astype(np.float16)

    consts = {
        "wcat1": wcat(W1, as1, ad1, 16),
        "wcat2": wcat(W2, as2, ad2, 128),
        "wcat3": wcat(W3, as3, ad3, 128),
        "iota": np.tile(np.arange(128, dtype=np.float16)[None, :], (128, 1)).copy(),
        "ones1": np.ones((128, 1), np.float16),
        "brep1": np.tile(b1[None, :].astype(np.float32), (128, 1)).copy(),
        "brep2": np.tile(b2[None, :].astype(np.float32), (128, 1)).copy(),
        "brep3": np.tile(b3[None, :].astype(np.float32), (128, 1)).copy(),
        "m1a": M1[:128].astype(np.float16),
        "m1b": M1[128:129].astype(np.float16),
        "m2": M2.astype(np.float16),
        "m3": np.ascontiguousarray(M3.astype(np.float16)),
        "mb1": mb1.reshape(128, 1).astype(np.float32),
        "mb2": mb2.reshape(64, 1).astype(np.float32),
        "mb3": mb3.reshape(2, 1).astype(np.float32),
        "bs": np.asarray(batch_size_tensor, np.float32).reshape(1, G).copy(),
    }
    cnt = np.bincount(batch, minlength=G).astype(np.float32)
    consts["rmean"] = (1.0 / np.maximum(cnt, 1.0)).reshape(G, 1).astype(np.float32)

    in_maps = []
    for k in range(C):
        m = dict(consts)
        m["xt1"] = xT
        m["idx_main"] = idx_main[k]
        m["idx_ed"] = idx_ed[k]
        m["dcol"] = dcol[k]
        m["pool1h"] = np.ascontiguousarray(pool1h[k].reshape(128, TL * G))
        in_maps.append(m)
    return in_maps, B, EPAD, NCH


def _build(B, EPAD, NCH):
    from concourse import bacc, mybir, tile
    import concourse.bass as bass
    from concourse import library_config
    from concourse import bass_isa
    from contextlib import ExitStack

    fp16 = mybir.dt.float16
    f32 = mybir.dt.float32
    i16 = mybir.dt.int16
    AF = mybir.ActivationFunctionType
    ALU = mybir.AluOpType

    nc = bacc.Bacc("TRN2", target_bir_lowering=False, debug=False)
    ctx = ExitStack()

    inp = {}
    for name, shape, dt in [
        ("xt1", [16, NP], fp16),
        ("idx_main", [128, EPAD // 16], i16),
        ("idx_ed", [128, EPAD // 16], i16),
        ("dcol", [128, TL * B], fp16),
        ("pool1h", [128, TL * G], fp16),
        ("wcat1", [16, 520], fp16),
        ("wcat2", [128, 520], fp16),
        ("wcat3", [128, 520], fp16),
        ("iota", [128, 128], fp16),
        ("ones1", [128, 1], fp16),
        ("brep1", [128, 128], f32),
        ("brep2", [128, 128], f32),
        ("brep3", [128, 128], f32),
        ("m1a", [128, 128], fp16),
        ("m1b", [1, 128], fp16),
        ("m2", [128, 64], fp16),
        ("m3", [64, 2], fp16),
        ("mb1", [128, 1], f32),
        ("mb2", [64, 1], f32),
        ("mb3", [2, 1], f32),
        ("bs", [1, G], f32),
        ("rmean", [G, 1], f32),
    ]:
        inp[name] = nc.dram_tensor(name, shape, dt, kind="ExternalInput")
    out_d = nc.dram_tensor("out", [2, G], f32, kind="ExternalOutput")

    table = nc.dram_tensor("htable", [NP, ROW], fp16, kind="Internal")
    edtab = nc.dram_tensor("edtable", [NP, ED_ROW], f32, kind="Internal")
    ag_in = nc.dram_tensor("ag_in", [SHP, HID], fp16, kind="Internal")
    ag_out = nc.dram_tensor(
        "ag_out", [NP, HID], fp16, kind="Internal", addr_space="Shared"
    )
    ar_in = nc.dram_tensor("ar_in", [G, HID], f32, kind="Internal")
    ar_out = nc.dram_tensor(
        "ar_out", [G, HID], f32, kind="Internal", addr_space="Shared"
    )
    pooled_d = nc.dram_tensor("pooled", [G, HID], fp16, kind="Internal")

    with tile.TileContext(nc) as tc:
        with (
            tc.tile_pool(name="const", bufs=1) as cpool,
            tc.tile_pool(name="xt", bufs=1) as xtpool,
            tc.tile_pool(name="stage", bufs=3) as stpool,
            tc.tile_pool(name="gath", bufs=2) as gpool,
            tc.tile_pool(name="edg", bufs=2) as epool,
            tc.tile_pool(name="small", bufs=3) as spool,
            tc.tile_pool(name="inc", bufs=2) as ipool,
            tc.tile_pool(name="inch", bufs=2) as hpool,
            tc.tile_pool(name="outs", bufs=2) as opool,
            tc.tile_pool(name="edst", bufs=1) as edstpool,
            tc.tile_pool(name="ps_h", bufs=1, space="PSUM") as ps_h,
            tc.tile_pool(name="ps_e", bufs=1, space="PSUM") as ps_e,
            tc.tile_pool(name="ps_a", bufs=2, space="PSUM") as ps_a,
        ):
            nc.gpsimd.load_library(library_config.mlp)

            cons = {}
            for name in [
                "idx_main", "idx_ed", "dcol", "pool1h", "wcat1", "wcat2",
                "wcat3", "iota", "ones1", "brep1", "brep2", "brep3", "m1a",
                "m1b", "m2", "m3", "mb1", "mb2", "mb3", "bs", "rmean",
            ]:
                t = cpool.tile(list(inp[name].shape), inp[name].dtype, tag=name)
                nc.sync.dma_start(t[:], inp[name][:])
                cons[name] = t

            xt1 = xtpool.tile([128, NP], fp16, tag="xt")
            nc.sync.dma_start(xt1[0:16, :], inp["xt1"][:])

            pool_ps = None
            for layer in range(1, 4):
                wc = cons[f"wcat{layer}"]
                brep = cons[f"brep{layer}"]
                if layer == 1:
                    xt = xt1
                    kdim = 16
                else:
                    xt = xtpool.tile([128, NP], fp16, tag="xt")
                    nc.sync.dma_start(xt[:], ag_out[:], transpose=True)
                    kdim = 128

                edstage = edstpool.tile([128, NT * 4], f32, tag="edst")
                esedmax = edstpool.tile([128, 8], f32, tag="esedmax")
                nc.vector.memset(esedmax[:], -1e30)
                # ---- prep h-table ----
                for nt in range(NT):
                    ph = ps_h.tile([128, 512], f32, tag="ph")
                    pe = ps_e.tile([128, 8], f32, tag="pe")
                    lhs = xt[:kdim, nt * 128 : (nt + 1) * 128]
                    nc.tensor.matmul(ph[:], lhs, wc[:kdim, 0:512])
                    nc.tensor.matmul(pe[:], lhs, wc[:kdim, 512:520])
                    st = stpool.tile([128, ROW], fp16, tag="stage")
                    # interleaved copy h -> cols h*129..h*129+128
                    nc.vector.tensor_copy(
                        st[:, 0:516].rearrange("p (h c) -> p h c", c=129)[
                            :, :, 0:128
                        ],
                        ph[:].rearrange("p (h c) -> p h c", h=4),
                    )
                    # ones cols at 129h+128
                    nc.vector.memset(
                        st[:, 0:516].rearrange("p (h c) -> p h c", c=129)[
                            :, :, 128:129
                        ],
                        1.0,
                    )
                    # es f32 at fp16 cols 516..524
                    nc.vector.tensor_copy(
                        st[:, 516:524].bitcast(f32), pe[:, 0:4]
                    )
                    nc.vector.tensor_copy(
                        edstage[:, nt * 4 : (nt + 1) * 4], pe[:, 4:8]
                    )
                    nc.vector.tensor_tensor(
                        esedmax[:], esedmax[:], pe[:, 0:8], ALU.max
                    )
                    eng = nc.sync if nt % 2 == 0 else nc.scalar
                    eng.dma_start(
                        table[nt * 128 : (nt + 1) * 128, 0:524], st[:, 0:524]
                    )
                for hh in range(2):
                    nc.gpsimd.dma_start(
                        edtab[:].rearrange("(t p) c -> p t c", p=128)[
                            :, hh * (NT // 2) : (hh + 1) * (NT // 2), 0:4
                        ],
                        edstage[:].rearrange("p (t c) -> p t c", c=4)[
                            :, hh * (NT // 2) : (hh + 1) * (NT // 2), :
                        ],
                    )

                redmax = edstpool.tile([128, 8], f32, tag="redmax")
                nc.gpsimd.partition_all_reduce(
                    redmax[:], esedmax[:], 128, bass_isa.ReduceOp.max
                )
                madd = edstpool.tile([128, 4], f32, tag="madd")
                nc.vector.tensor_tensor(
                    madd[:], redmax[:, 0:4], redmax[:, 4:8], ALU.add
                )
                nc.vector.tensor_scalar(madd[:], madd[:], 0.0, None, ALU.max)

                # ---- edge phase ----
                otiles = {}
                for ch in range(NCH):
                    g = gpool.tile([128, CH_BLK, ROW], fp16, tag="g")
                    nc.gpsimd.dma_gather(
                        g[:], table[:],
                        cons["idx_main"][:, ch * (CH_BLK * 8) : (ch + 1) * (CH_BLK * 8)],
                        CH_BLK * 128, CH_BLK * 128, ROW,
                    )
                    ed = epool.tile([128, CH_BLK, ED_ROW], f32, tag="ed")
                    nc.gpsimd.dma_gather(
                        ed[:], edtab[:],
                        cons["idx_ed"][:, ch * (CH_BLK * 8) : (ch + 1) * (CH_BLK * 8)],
                        CH_BLK * 128, CH_BLK * 128, ED_ROW,
                    )
                    logit = spool.tile([128, CH_BLK * 4], f32, tag="logit")
                    es_v = g[:].rearrange("p b r -> p b r")[:, :, 516:524].bitcast(f32)
                    nc.vector.tensor_tensor(
                        logit[:].rearrange("p (b h) -> p b h", h=4),
                        es_v, ed[:, :, 0:4], ALU.add,
                    )
                    lrl = spool.tile([128, CH_BLK * 4], f32, tag="lrl")
                    nc.scalar.activation(lrl[:], logit[:], AF.Lrelu, alpha=0.2)
                    nc.vector.tensor_tensor(
                        lrl[:].rearrange("p (b h) -> p b h", h=4),
                        lrl[:].rearrange("p (b h) -> p b h", h=4),
                        madd[:].unsqueeze(1).broadcast_to([128, CH_BLK, 4]),
                        ALU.subtract,
                    )
                    w16 = spool.tile([128, CH_BLK * 4], fp16, tag="w16")
                    nc.scalar.activation(w16[:], lrl[:], AF.Exp)

                    inc = ipool.tile([128, CH_BLK * 128], fp16, tag="inc")
                    dc = cons["dcol"][:, ch * CH_BLK : (ch + 1) * CH_BLK]
                    nc.vector.tensor_tensor(
                        inc[:].rearrange("p (b c) -> p b c", c=128),
                        dc.unsqueeze(2).broadcast_to([128, CH_BLK, 128]),
                        cons["iota"][:].unsqueeze(1).broadcast_to(
                            [128, CH_BLK, 128]
                        ),
                        ALU.is_equal,
                    )
                    inchs = []
                    for h in range(HEADS):
                        ih = hpool.tile([128, CH_BLK * 128], fp16, tag=f"ih{h}")
                        wv = w16[:].rearrange("p (b h) -> p b h", h=4)[:, :, h]
                        nc.vector.tensor_tensor(
                            ih[:].rearrange("p (b c) -> p b c", c=128),
                            inc[:].rearrange("p (b c) -> p b c", c=128),
                            wv.unsqueeze(2).broadcast_to([128, CH_BLK, 128]),
                            ALU.mult,
                        )
                        inchs.append(ih)

                    for blk in range(CH_BLK):
                        j = ch * CH_BLK + blk
                        t_id, pos = j // B, j % B
                        if pos == 0:
                            otiles[t_id] = ps_a.tile([128, 4, 256], f32, tag="agg", name=f"agg{t_id%4}")
                        pa = otiles[t_id]
                        for h in range(HEADS):
                            nc.tensor.matmul(
                                pa[:, h, 0:129],
                                inchs[h][:, blk * 128 : (blk + 1) * 128],
                                g[:, blk, h * 129 : h * 129 + 129],
                                start=(pos == 0),
                                stop=(pos == B - 1),
                                skip_group_check=True,
                            )
                        if pos == B - 1:
                            self_t = t_id
                            pa = otiles.pop(self_t)
                            den = opool.tile([128, 4], f32, tag="den")
                            nc.vector.tensor_scalar(
                                den[:], pa[:, :, 128], 1e-30, None, ALU.max
                            )
                            rden = opool.tile([128, 4], f32, tag="rden")
                            nc.vector.reciprocal(rden[:], den[:])
                            hm = opool.tile([128, 4 * 128], f32, tag="hm")
                            for h in range(HEADS):
                                nc.scalar.activation(
                                    hm[:, h * 128 : (h + 1) * 128],
                                    pa[:, h, 0:128],
                                    AF.Copy,
                                    scale=rden[:, h : h + 1],
                                )
                            s0 = opool.tile([128, 128], f32, tag="s0")
                            nc.vector.tensor_tensor(
                                s0[:], hm[:, 0:128], hm[:, 128:256], ALU.add
                            )
                            s1 = opool.tile([128, 128], f32, tag="s1")
                            nc.vector.tensor_tensor(
                                s1[:], hm[:, 256:384], hm[:, 384:512], ALU.add
                            )
                            s2 = opool.tile([128, 128], f32, tag="s2")
                            nc.vector.tensor_tensor(s2[:], s0[:], s1[:], ALU.add)
                            s3 = opool.tile([128, 128], f32, tag="s3")
                            nc.vector.tensor_tensor(s3[:], s2[:], brep[:], ALU.add)
                            ro = opool.tile([128, 128], fp16, tag="ro")
                            # relu + 0.25 head-mean + fp16
                            nc.scalar.activation(ro[:], s3[:], AF.Relu, scale=0.25)
                            if layer < 3:
                                nc.sync.dma_start(
                                    ag_in[self_t * 128 : (self_t + 1) * 128, :],
                                    ro[:],
                                )
                            else:
                                if pool_ps is None:
                                    pool_ps = ps_h.tile([64, 128], f32, tag="poolps", name="poolps")
                                nc.tensor.matmul(
                                    pool_ps[:],
                                    cons["pool1h"][
                                        :, self_t * G : (self_t + 1) * G
                                    ],
                                    ro[:],
                                    start=(self_t == 0),
                                    stop=(self_t == TL - 1),
                                    skip_group_check=True,
                                )
                if layer < 3:
                    nc.gpsimd.collective_compute(
                        "AllGather",
                        ALU.bypass,
                        replica_groups=[list(range(C))],
                        ins=[ag_in[:].opt()],
                        outs=[ag_out[:].opt()],
                    )

            # ---- pooling + MLP ----
            sums = opool.tile([64, 128], f32, tag="sums")
            nc.vector.tensor_copy(sums[:], pool_ps[:])
            nc.sync.dma_start(ar_in[:], sums[:])
            nc.gpsimd.collective_compute(
                "AllReduce",
                ALU.add,
                replica_groups=[list(range(C))],
                ins=[ar_in[:].opt()],
                outs=[ar_out[:].opt()],
            )
            asums = opool.tile([64, 128], f32, tag="asums")
            nc.sync.dma_start(asums[:], ar_out[:])
            pooled = opool.tile([64, 128], fp16, tag="pooled")
            nc.vector.tensor_scalar(
                pooled[:], asums[:], cons["rmean"][:, 0:1], None, ALU.mult
            )
            nc.sync.dma_start(pooled_d[:], pooled[:])
            pooledT = opool.tile([128, 64], fp16, tag="pooledT")
            nc.sync.dma_start(pooledT[:], pooled_d[:], transpose=True)

            lbs = opool.tile([1, G], fp16, tag="lbs")
            nc.scalar.activation(lbs[:], cons["bs"][:], AF.Ln, bias=1.0)
            z1p = ps_e.tile([128, 64], f32, tag="pe")
            nc.tensor.matmul(
                z1p[:], cons["m1a"][:], pooledT[:], start=True, stop=False,
                skip_group_check=True,
            )
            nc.tensor.matmul(
                z1p[:], cons["m1b"][:], lbs[:], start=False, stop=True,
                skip_group_check=True,
            )
            z1 = opool.tile([128, 64], fp16, tag="z1")
            nc.scalar.activation(z1[:], z1p[:], AF.Relu, bias=cons["mb1"][:, 0:1])
            z2p = ps_e.tile([64, 64], f32, tag="pe")
            nc.tensor.matmul(z2p[:], cons["m2"][:], z1[:])
            z2 = opool.tile([64, 64], fp16, tag="z2")
            nc.scalar.activation(z2[:], z2p[:], AF.Relu, bias=cons["mb2"][:, 0:1])
            z3p = ps_e.tile([2, 64], f32, tag="pe")
            nc.tensor.matmul(z3p[:], cons["m3"][:], z2[:])
            zo = opool.tile([2, 64], f32, tag="zo")
            nc.vector.tensor_scalar(
                zo[:], z3p[:], cons["mb3"][:, 0:1], None, ALU.add
            )
            nc.sync.dma_start(out_d[:], zo[:])

    nc.compile()
    ctx.close()
    return nc


def kernel(x, edge_index, batch, batch_size_tensor,
           W1, as1, ad1, b1, W2, as2, ad2, b2, W3, as3, ad3, b3,
           M1, mb1, M2, mb2, M3, mb3, _trace=False):
    x = np.asarray(x, np.float32)
    edge_index = np.asarray(edge_index)
    batch = np.asarray(batch, np.int64)
    in_maps, B, EPAD, NCH = _host_prep(
        x, edge_index, batch, np.asarray(batch_size_tensor, np.float32),
        [(W1, as1, ad1, b1), (W2, as2, ad2, b2), (W3, as3, ad3, b3)],
        [M1, mb1, M2, mb2, M3, mb3],
    )
    nc = _build(B, EPAD, NCH)
    from concourse.bass_utils import run_bass_kernel_spmd

    res = run_bass_kernel_spmd(nc, in_maps, core_ids=list(range(C)))
    out = res.results[0]["out"]
    if _trace:
        import time as _time

        for _ in range(2):  # warm
            run_bass_kernel_spmd(nc, in_maps, core_ids=list(range(C)))
        t0 = _time.time()
        nrep = 10
        for _ in range(nrep):
            run_bass_kernel_spmd(nc, in_maps, core_ids=list(range(C)))
        kernel.last_exec_ns = (_time.time() - t0) / nrep * 1e9
    return np.ascontiguousarray(out.T.astype(np.float32))
